# revision 19
# baseline (speedup 1.0000x reference)
"""Maxish pooling kernel for Trainium2 (8 NeuronCores, data-parallel).

Reference math (per row of length N):
    m  = max(x)
    rt = (x - m) / (m + 1e-8)
    pos = m * sum(exp(2*rt)) / sum(exp(rt))    # softmax identity (s == 1)
    out = m > 0 ? pos : (m < 0 ? m : 0)

Implementation trick: with v = exp(x * r), r = 1/(m+eps),
    pos = m * e^{-m r} * sum(v^2) / sum(v)  and  e^{-m r} = 1/e (eps tiny)
so the ACT exp needs no per-tile bias operand; 1/e is folded into the
epilogue.

Engine split per [128, 256] tile (rows on partitions):
  DVE:  per-row max (fold tree: one f32 fold then fp16 folds in 2x mode,
        short tensor_reduce tail), paired bn_stats over v (one
        instruction covers TWO tiles via an interleaved stream view:
        even/odd accumulator groups = tile A/B)
  ACT:  one exp per tile (per-partition scale); the last `asq` tiles of
        each full chunk use exp+accum / square+accum so ACT carries part
        of the sum work
  GpSimd: small per-chunk stat fixups
The first 16 tiles go through four 4-tile mini-chunks to shorten the
pipeline ramp, and the epilogue + store run per 128-column block so the
tail overlaps the last chunks.
"""

import numpy as np

P = 128
N = 256
SMALL = 1e-8
INV_E = 0.36787944117144233


def _build(n_rows: int, s: float, G: int = 16, x_bufs: int = 4,
           u_bufs: int = 3, asq: int = 2, seg: int = 2, fold: int = 3,
           fast_recip: bool = True, smalls_gpsimd: bool = True,
           gps2: bool = False, pro: int = 0):
    from concourse import bacc, mybir
    from concourse import masks
    from concourse.tile import TileContext

    f32 = mybir.dt.float32
    f16 = mybir.dt.float16
    Act = mybir.ActivationFunctionType
    Alu = mybir.AluOpType
    Ax = mybir.AxisListType

    assert n_rows % (P * G) == 0
    T = n_rows // P          # tiles of [128, N]
    fast = (s == 1.0)

    # chunk schedule: a few small chunks first to get ACT going early
    chunks = []
    t0 = 0
    if pro and fast:
        for _ in range(G // pro):
            chunks.append((t0, pro))
            t0 += pro
    while t0 < T:
        chunks.append((t0, G))
        t0 += G

    nc = bacc.Bacc("TRN2", target_bir_lowering=False, debug=False,
                   num_devices=8)
    x_d = nc.declare_dram_parameter("x", [n_rows, N], f32, isOutput=False)
    out_d = nc.declare_dram_parameter("out", [n_rows], f32, isOutput=True)

    with TileContext(nc) as tc:
        with (
            tc.tile_pool(name="xp", bufs=x_bufs) as xp,
            tc.tile_pool(name="fp", bufs=2) as fp,
            tc.tile_pool(name="up", bufs=u_bufs) as up,
            tc.tile_pool(name="stat", bufs=1) as statp,
            tc.tile_pool(name="consts", bufs=4) as cpool,
            tc.tile_pool(name="psum", bufs=2, space="PSUM") as psp,
        ):
            M = statp.tile([P, T], f32, tag="M")       # per-row max
            S1 = statp.tile([P, T], f32, tag="S1")     # sum v^2
            S2 = statp.tile([P, T], f32, tag="S2")     # sum v
            R = statp.tile([P, T], f32, tag="R")       # final per-row result
            RT = statp.tile([P, T], f32, tag="RT")     # transposed result
            MK = statp.tile([P, T], mybir.dt.uint8, tag="MK")  # m>0 mask

            ident = statp.tile([P, P], f32, tag="ident")
            masks.make_identity(nc, ident[:])

            sm = nc.gpsimd if smalls_gpsimd else nc.vector
            sm2 = nc.gpsimd if gps2 else nc.vector

            def epilogue(k):
                """Finish columns [k*P, (k+1)*P) of the stats and store."""
                cs = slice(k * P, (k + 1) * P)
                if fast_recip:
                    nc.vector.reciprocal_approx_fast(S2[:, cs], S2[:, cs])
                else:
                    nc.vector.reciprocal(S2[:, cs], S2[:, cs])
                nc.vector.tensor_tensor(S1[:, cs], S1[:, cs], S2[:, cs],
                                        op=Alu.mult)
                if fast:
                    nc.vector.scalar_tensor_tensor(
                        out=S1[:, cs], in0=S1[:, cs], scalar=INV_E,
                        in1=M[:, cs], op0=Alu.mult, op1=Alu.mult)
                else:
                    nc.vector.tensor_tensor(S1[:, cs], S1[:, cs], M[:, cs],
                                            op=Alu.mult)
                # out = m > 0 ? pos : (m < 0 ? m : 0); uint8 mask for
                # CopyPredicated
                nc.vector.tensor_scalar(MK[:, cs], M[:, cs], 0.0, None,
                                        op0=Alu.is_gt)
                nc.vector.tensor_copy(R[:, cs], M[:, cs])
                nc.vector.copy_predicated(out=R[:, cs], mask=MK[:, cs],
                                          data=S1[:, cs])
                # transpose R block [128, 128] so the store DMA has 512B
                # runs: out row = t*128 + p = (k*128 + t_lo)*128 + p
                pt = psp.tile([P, P], f32, tag="pt")
                nc.tensor.transpose(pt[:], R[:, cs], ident[:])
                nc.vector.tensor_copy(RT[:, cs], pt[:])
                nc.sync.dma_start(
                    out=out_d[k * P * P:(k + 1) * P * P].rearrange(
                        "(t p) -> t p", p=P),
                    in_=RT[:, cs])

            for ci, (t0, gc) in enumerate(chunks):
                # asq in steps of 2 (pairing needs ga even); odd values
                # alternate asq+1 / asq-1 per chunk to hit the average
                if asq % 2 == 0:
                    aq = asq
                else:
                    aq = asq + 1 if ci % 2 == 0 else asq - 1
                aq = aq if (fast and gc == G) else 0
                ga = gc - aq         # tiles whose sums go through bn_stats
                xt = xp.tile([P, G * N], f32, tag="x")
                src = x_d[t0 * P:(t0 + gc) * P, :].rearrange(
                    "(g p) n -> p g n", p=P)
                nc.sync.dma_start(
                    out=xt[:, :gc * N].rearrange("p (g n) -> p g n", n=N),
                    in_=src)

                x3 = xt[:, :gc * N].rearrange("p (g n) -> p g n", n=N)
                mg = M[:, t0:t0 + gc]
                if fold:
                    # max fold tree: one f32 fold (halves the 1x reduce
                    # stream), then fp16 folds at 2x, short reduce tail
                    H = N // 2
                    ht = fp.tile([P, G * H], f16, tag="h")
                    h3 = ht[:, :gc * H].rearrange("p (g h) -> p g h", h=H)
                    nc.vector.tensor_tensor(
                        h3, x3[:, :, 0:H], x3[:, :, H:N], op=Alu.max)
                    w = H
                    for _ in range(1, fold):
                        nc.vector.tensor_tensor(
                            h3[:, :, 0:w // 2], h3[:, :, 0:w // 2],
                            h3[:, :, w // 2:w], op=Alu.max)
                        w //= 2
                    nc.vector.tensor_reduce(out=mg, in_=h3[:, :, 0:w],
                                            axis=Ax.X, op=Alu.max)
                else:
                    nc.vector.tensor_reduce(out=mg, in_=x3, axis=Ax.X,
                                            op=Alu.max)
                # per-chunk consts in a versioned pool tile so ACT's reads
                # of chunk c don't serialize against DVE writing chunk c+1
                cb = cpool.tile([P, 4 * G], f32, tag="cb")
                rg = cb[:, 0:gc]
                # rg = 1 / (m + eps), clamped to >= 0 so the exponent x*r
                # stays <= m*r ~ 1 (m<0 rows get r=0 -> v=1, masked later)
                nc.vector.tensor_scalar_add(rg, mg, SMALL)
                if fast_recip:
                    nc.vector.reciprocal_approx_fast(rg, rg)
                else:
                    nc.vector.reciprocal(rg, rg)
                nc.vector.tensor_scalar_max(rg, rg, 0.0)

                if fast:
                    ut = up.tile([P, G * N], f16, tag="u")
                    for g in range(ga):
                        fs = slice(g * N, (g + 1) * N)
                        nc.scalar.activation(
                            out=ut[:, fs], in_=xt[:, fs], func=Act.Exp,
                            scale=rg[:, g:g + 1])
                    for g in range(ga, gc):
                        fs = slice(g * N, (g + 1) * N)
                        j = t0 + g
                        nc.scalar.activation(
                            out=ut[:, fs], in_=xt[:, fs], func=Act.Exp,
                            scale=rg[:, g:g + 1], accum_out=S2[:, j:j + 1])
                        nc.scalar.activation(
                            out=ut[:, fs], in_=ut[:, fs], func=Act.Square,
                            accum_out=S1[:, j:j + 1])
                    if seg == 2 and ga:
                        # one bn_stats per TWO tiles: the [P, n, 2] view
                        # streams A0,B0,A1,B1,... so the hardware's
                        # even/odd accumulator groups hold tile A / tile B
                        # (out stays [P, 6] as the verifier requires;
                        # emitted directly because the bass wrapper only
                        # allows [P, N] here)
                        npr = ga // 2
                        bst = cpool.tile([P, (G // 2) * 6], f32, tag="bst")
                        for j in range(npr):
                            pv = ut[:, 2 * j * N:(2 * j + 2) * N].rearrange(
                                "p (b n) -> p n b", b=2)
                            ov = bst[:, j * 6:(j + 1) * 6]
                            nc.vector.add_instruction(
                                mybir.InstBNStats(
                                    name=nc.vector.bass
                                    .get_next_instruction_name(),
                                    ins=[nc.vector.lower_ap(pv)],
                                    outs=[nc.vector.lower_ap(ov)]))
                        bsg = bst[:, :npr * 6].rearrange(
                            "p (j s) -> p s j", s=6)
                        mu_e, m2_e = bsg[:, 1], bsg[:, 2]
                        mu_o, m2_o = bsg[:, 4], bsg[:, 5]
                        s2c = S2[:, t0:t0 + ga].rearrange(
                            "p (j b) -> p b j", b=2)
                        s1c = S1[:, t0:t0 + ga].rearrange(
                            "p (j b) -> p b j", b=2)
                        t2 = cb[:, G:G + npr]
                        t3 = cb[:, 2 * G:2 * G + npr]
                        fn = float(N)
                        sm2.tensor_scalar_mul(s2c[:, 0], mu_e, fn)
                        sm2.tensor_scalar_mul(s2c[:, 1], mu_o, fn)
                        sm.tensor_tensor(t2, mu_e, mu_e, op=Alu.mult)
                        sm.tensor_tensor(t3, mu_o, mu_o, op=Alu.mult)
                        nc.vector.scalar_tensor_tensor(
                            out=s1c[:, 0], in0=t2, scalar=fn, in1=m2_e,
                            op0=Alu.mult, op1=Alu.add)
                        nc.vector.scalar_tensor_tensor(
                            out=s1c[:, 1], in0=t3, scalar=fn, in1=m2_o,
                            op0=Alu.mult, op1=Alu.add)
                    elif ga:
                        bst = cpool.tile([P, G * 6], f32, tag="bstu")
                        for g in range(ga):
                            nc.vector.bn_stats(
                                out=bst[:, g * 6:(g + 1) * 6],
                                in_=ut[:, g * N:(g + 1) * N])
                        bsg = bst[:, :ga * 6].rearrange(
                            "p (g s) -> p s g", s=6)
                        mu_e, m2_e = bsg[:, 1], bsg[:, 2]
                        mu_o, m2_o = bsg[:, 4], bsg[:, 5]
                        s2c = S2[:, t0:t0 + ga]
                        s1c = S1[:, t0:t0 + ga]
                        t1 = cb[:, G:G + ga]
                        t2 = cb[:, 2 * G:2 * G + ga]
                        t3 = cb[:, 3 * G:3 * G + ga]
                        half = float(N // 2)
                        sm.tensor_tensor(t1, mu_e, mu_o, op=Alu.add)
                        nc.vector.tensor_scalar_mul(s2c, t1, half)
                        sm.tensor_tensor(t2, mu_e, mu_e, op=Alu.mult)
                        sm.tensor_tensor(t3, mu_o, mu_o, op=Alu.mult)
                        sm.tensor_tensor(t2, t2, t3, op=Alu.add)
                        sm.tensor_tensor(t1, m2_e, m2_o, op=Alu.add)
                        nc.vector.scalar_tensor_tensor(
                            out=s1c, in0=t2, scalar=half, in1=t1,
                            op0=Alu.mult, op1=Alu.add)
                else:
                    bg = cb[:, G:2 * G]
                    c1 = cb[:, 2 * G:3 * G]
                    b1 = cb[:, 3 * G:4 * G]
                    nc.vector.scalar_tensor_tensor(
                        out=bg[:, :gc], in0=mg, scalar=-1.0, in1=rg,
                        op0=Alu.mult, op1=Alu.mult)
                    nc.vector.tensor_scalar_mul(c1[:, :gc], rg, 1.0 + s)
                    nc.vector.tensor_scalar_mul(b1[:, :gc], bg[:, :gc],
                                                1.0 + s)
                    nc.vector.tensor_scalar_mul(rg, rg, s)
                    nc.vector.tensor_scalar_mul(bg[:, :gc], bg[:, :gc], s)
                    ut = up.tile([P, G * N], f32, tag="uf")
                    for g in range(gc):
                        fs = slice(g * N, (g + 1) * N)
                        j = t0 + g
                        nc.scalar.activation(
                            out=ut[:, fs], in_=xt[:, fs], func=Act.Exp,
                            scale=rg[:, g:g + 1], bias=bg[:, g:g + 1],
                            accum_out=S2[:, j:j + 1])
                        nc.scalar.activation(
                            out=ut[:, fs], in_=xt[:, fs], func=Act.Exp,
                            scale=c1[:, g:g + 1], bias=b1[:, g:g + 1],
                            accum_out=S1[:, j:j + 1])

            for k in range(T // P):
                epilogue(k)

    nc.compile()
    return nc


def _run(x: np.ndarray, scale: np.ndarray, trace: bool = False,
         build_kw: dict | None = None, **kw):
    from concourse.bass_utils import run_bass_kernel_spmd

    n_cores = 8
    B, Tm, X, Nn = x.shape          # 32, 256, 64, 256
    assert Nn == N
    rows = B * Tm * X
    rows_per_core = rows // n_cores
    s = float(np.asarray(scale))

    nc = _build(rows_per_core, s, **(build_kw or {}))
    xs = np.ascontiguousarray(np.asarray(x, dtype=np.float32)).reshape(
        n_cores, rows_per_core, N)
    in_maps = [{"x": xs[i]} for i in range(n_cores)]
    res = run_bass_kernel_spmd(nc, in_maps, list(range(n_cores)),
                               trace=trace, **kw)
    out = np.concatenate([r["out"].reshape(-1) for r in res.results], axis=0)
    return out.reshape(B, Tm, X).astype(np.float32), res


def kernel(x: np.ndarray, scale: np.ndarray) -> np.ndarray:
    return _run(x, scale)[0]
